# revision 13
# baseline (speedup 1.0000x reference)
import numpy as np

# nn_NearestNeighbours: batch [8,512,512] f32, emb [50000,512] f32,
# output argmin indices [8,512] int32. Vocab-sharded across 8 cores.
# Screen: fp8e4m3 DoubleRow GEMM in 4 psum groups of 1536 + tail 128.
# Evictions to f16 SBUF: DVE takes an early slice of group 0, ACT the
# rest. DVE group-elementwise max tree: L1 G0vG1, G2vG3 -> L2 -> quad
# [1536] -> L3 [768] -> L4 [384]; tail copied straight into the output
# tile. Host picks global top-K of the shipped group maxima, expands
# each group to its 16 members (j + 384a + 1536b) and rescores exactly
# in f32 cosine.
B, S, E, V = 8, 512, 512, 50000
R = B * S              # 4096 token rows
NC = 8                 # cores
VS = V // NC           # 6250 vocab rows per core
VSP = 6272             # 4*1536 + 128 tail
GW = 1536              # psA group width
NG = 4                 # psA groups
TAIL = 128             # psB tail width (106 genuine + 22 pad)
DVE_CUT = 1376         # group-0 prefix evicted by DVE instead of ACT
GM = 512               # 384 quad-tree maxima + 128 raw tail columns
TOPK = 20              # groups rescored exactly on host

_CACHE = {}


def _build():
    import concourse.bacc as bacc
    import concourse.mybir as mybir
    from concourse.tile import TileContext

    dtf = mybir.dt.float32
    dt8 = mybir.dt.float8e4
    dth = mybir.dt.float16
    DR = mybir.MatmulPerfMode.DoubleRow

    nc = bacc.Bacc("TRN2", target_bir_lowering=False, debug=False)
    bT_ap = nc.dram_tensor("bT", [E, R], dt8, kind="ExternalInput").ap()
    embT_ap = nc.dram_tensor("embT", [E, VSP], dt8, kind="ExternalInput").ap()
    gm_ap = nc.dram_tensor("gm", [R, GM], dth, kind="ExternalOutput").ap()

    KT = E // 128
    MT = R // 128
    with TileContext(nc) as tc:
        with tc.sbuf_pool(name="emb", bufs=1) as embp, \
             tc.sbuf_pool(name="bt", bufs=2) as btp, \
             tc.sbuf_pool(name="sc", bufs=2) as scp, \
             tc.sbuf_pool(name="tr", bufs=2) as trp, \
             tc.sbuf_pool(name="out", bufs=2) as outp, \
             tc.psum_pool(name="psA", bufs=2) as psA, \
             tc.psum_pool(name="psB", bufs=2) as psB:
            gt = btp.tile([128, KT, 512], dt8)
            for k in range(KT):
                nc.scalar.dma_start(gt[:, k:k + 1, :],
                                    bT_ap[128 * k:128 * (k + 1), 0:512])
            emb8 = embp.tile([128, KT, VSP], dt8, name="emb8")
            off = 0
            for w in [1024] * 6 + [TAIL]:
                eng = nc.scalar if off >= 5120 else nc.sync
                for k in range(KT):
                    eng.dma_start(
                        emb8[:, k:k + 1, off:off + w],
                        embT_ap[128 * k:128 * (k + 1), off:off + w],
                    )
                off += w

            for g in range(MT // 4):
                cur = gt
                if g + 1 < MT // 4:
                    gt = btp.tile([128, KT, 512], dt8)
                    for k in range(KT):
                        nc.sync.dma_start(
                            gt[:, k:k + 1, :],
                            bT_ap[128 * k:128 * (k + 1), 512 * (g + 1):512 * (g + 2)],
                        )
                for mm in range(4):
                    m = g * 4 + mm
                    sc = scp.tile([128, NG * GW], dth)
                    gmo = outp.tile([128, GM], dth)
                    # tail MMs first: fillers that need no psA slot
                    ptb = psB.tile([128, TAIL], dtf)
                    for p in range(2):
                        nc.tensor.matmul(
                            ptb[:],
                            cur[:, 2 * p:2 * p + 2, 128 * mm:128 * mm + 128],
                            emb8[:, 2 * p:2 * p + 2, NG * GW:VSP],
                            start=(p == 0),
                            stop=(p == 1),
                            perf_mode=DR,
                        )
                    nc.scalar.copy(gmo[:, 384:GM], ptb[:])
                    for gi in range(NG):
                        off = gi * GW
                        pt = psA.tile([128, GW], dtf)
                        for p in range(2):
                            for c0 in range(0, GW, 512):
                                nc.tensor.matmul(
                                    pt[:, c0:c0 + 512],
                                    cur[:, 2 * p:2 * p + 2, 128 * mm:128 * mm + 128],
                                    emb8[:, 2 * p:2 * p + 2,
                                         off + c0:off + c0 + 512],
                                    start=(p == 0),
                                    stop=(p == 1),
                                    perf_mode=DR,
                                )
                        if gi == 0:
                            # early DVE slice balances the engines without
                            # delaying this slot's release
                            nc.vector.tensor_copy(sc[:, 0:DVE_CUT],
                                                  pt[:, 0:DVE_CUT])
                            nc.scalar.copy(sc[:, DVE_CUT:GW], pt[:, DVE_CUT:GW])
                        else:
                            nc.scalar.copy(sc[:, off:off + GW], pt[:])
                    t1a = trp.tile([128, GW], dth)
                    nc.vector.tensor_max(t1a[:], sc[:, 0:GW], sc[:, GW:2 * GW])
                    t1b = trp.tile([128, GW], dth)
                    nc.vector.tensor_max(t1b[:], sc[:, 2 * GW:3 * GW],
                                         sc[:, 3 * GW:4 * GW])
                    q = trp.tile([128, GW], dth)
                    nc.vector.tensor_max(q[:], t1a[:], t1b[:])
                    t3 = trp.tile([128, 768], dth)
                    nc.vector.tensor_max(t3[:], q[:, 0:768], q[:, 768:GW])
                    nc.vector.tensor_max(gmo[:, 0:384], t3[:, 0:384],
                                         t3[:, 384:768])
                    nc.sync.dma_start(gm_ap[128 * m:128 * (m + 1), :], gmo[:])
    nc.compile()
    return nc


def _run(batch: np.ndarray, emb: np.ndarray, trace: bool = False, **kw):
    import ml_dtypes
    from concourse import bass_utils

    if "nc" not in _CACHE:
        _CACHE["nc"] = _build()
    nc = _CACHE["nc"]
    f8 = ml_dtypes.float8_e4m3

    b = np.ascontiguousarray(batch.reshape(R, E).astype(np.float32))
    bT8 = np.ascontiguousarray(b.T).astype(f8)
    embT8 = emb.T.astype(f8)
    in_maps = []
    for c in range(NC):
        shardT = np.zeros((E, VSP), f8)
        shardT[:, :VS] = embT8[:, c * VS:(c + 1) * VS]
        in_maps.append({"bT": bT8, "embT": shardT})

    res = bass_utils.run_bass_kernel_spmd(
        nc, in_maps, core_ids=list(range(NC)), trace=trace, **kw
    )

    # gm: [R, NC*512] f16. Per core block: entries 0..383 are quad-tree
    # group maxima (group j covers local ids j + 384a + 1536b, a,b<4);
    # entries 384..511 are raw tail columns (local id 6144 + (p-384)).
    gm = np.concatenate([res.results[c]["gm"] for c in range(NC)], axis=1)
    gm = gm.astype(np.float32)                                     # [R, 4096]

    top = np.argpartition(-gm, TOPK, axis=1)[:, :TOPK]             # [R,K]
    core = top // GM
    p = top - core * GM                                            # [R,K]
    is_grp = p < 384
    ab = (384 * np.arange(4)[:, None] + 1536 * np.arange(4)[None, :]).reshape(-1)
    grp_cand = p[:, :, None] + ab[None, None, :]                   # [R,K,16]
    tail_cand = (6144 + (p - 384))[:, :, None] * np.ones(16, np.int64)
    loc = np.where(is_grp[:, :, None], grp_cand, tail_cand).reshape(R, -1)
    corex = np.repeat(core, 16, axis=1)
    valid = loc < VS
    gid = np.where(valid, corex * VS + np.minimum(loc, VS - 1), 0)

    # exact rescore in f32 on normalized embeddings
    en = emb / np.sqrt((emb * emb).sum(axis=1, keepdims=True))
    bn = b / np.sqrt((b * b).sum(axis=1, keepdims=True))
    best = np.empty(R, np.int64)
    CH = 256
    for r0 in range(0, R, CH):
        r1 = min(r0 + CH, R)
        g = gid[r0:r1]
        ce = en[g]                                   # [ch,K*16,512]
        s = np.matmul(ce, bn[r0:r1, :, None])[:, :, 0]
        s[~valid[r0:r1]] = -np.inf
        am = np.argmax(s, axis=1)
        best[r0:r1] = g[np.arange(r1 - r0), am]

    return best.astype(np.int32).reshape(B, S), res


def kernel(batch: np.ndarray, emb: np.ndarray) -> np.ndarray:
    out, _ = _run(batch, emb, trace=False)
    return out


# revision 17
# speedup vs baseline: 1.0264x; 1.0264x over previous
import numpy as np

# nn_NearestNeighbours: batch [8,512,512] f32, emb [50000,512] f32,
# output argmin indices [8,512] int32. Vocab-sharded across 8 cores.
# Screen: fp8e4m3 DoubleRow GEMM in 4 psum groups of 1536 + tail 128.
# Evictions to f16 SBUF: DVE takes an early slice of group 0, ACT the
# rest. DVE group-elementwise max tree: L1 G0vG1, G2vG3 -> L2 -> quad
# [1536] -> L3 [768] -> L4 [384]; tail copied straight into the output
# tile. Host picks global top-K of the shipped group maxima, expands
# each group to its 16 members (j + 384a + 1536b) and rescores exactly
# in f32 cosine.
B, S, E, V = 8, 512, 512, 50000
R = B * S              # 4096 token rows
NC = 8                 # cores
VS = V // NC           # 6250 vocab rows per core
VSP = 6272             # 4*1536 + 128 tail
GW = 1536              # psA group width
NG = 4                 # psA groups
TAIL = 128             # psB tail width (106 genuine + 22 pad)
DVE_CUT = 1376         # group-0 prefix evicted by DVE instead of ACT
GM = 512               # 384 quad-tree maxima + 128 raw tail columns
TOPK = 20              # groups rescored exactly on host

_CACHE = {}


def _build():
    import concourse.bacc as bacc
    import concourse.mybir as mybir
    from concourse.tile import TileContext

    dtf = mybir.dt.float32
    dt8 = mybir.dt.float8e4
    dth = mybir.dt.float16
    DR = mybir.MatmulPerfMode.DoubleRow

    nc = bacc.Bacc("TRN2", target_bir_lowering=False, debug=False)
    bT_ap = nc.dram_tensor("bT", [E, R], dt8, kind="ExternalInput").ap()
    embT_ap = nc.dram_tensor("embT", [E, VSP], dt8, kind="ExternalInput").ap()
    gm_ap = nc.dram_tensor("gm", [R, GM], dth, kind="ExternalOutput").ap()

    KT = E // 128
    MT = R // 128
    with TileContext(nc) as tc:
        with tc.sbuf_pool(name="emb", bufs=1) as embp, \
             tc.sbuf_pool(name="bt", bufs=2) as btp, \
             tc.sbuf_pool(name="sc", bufs=2) as scp, \
             tc.sbuf_pool(name="tr", bufs=2) as trp, \
             tc.sbuf_pool(name="out", bufs=4) as outp, \
             tc.psum_pool(name="psA", bufs=2) as psA, \
             tc.psum_pool(name="psB", bufs=2) as psB:
            gt = btp.tile([128, KT, 512], dt8)
            for k in range(KT):
                nc.scalar.dma_start(gt[:, k:k + 1, :],
                                    bT_ap[128 * k:128 * (k + 1), 0:512])
            emb8 = embp.tile([128, KT, VSP], dt8, name="emb8")
            off = 0
            for w in [1024] * 6 + [TAIL]:
                eng = nc.scalar if off >= 5120 else nc.sync
                for k in range(KT):
                    eng.dma_start(
                        emb8[:, k:k + 1, off:off + w],
                        embT_ap[128 * k:128 * (k + 1), off:off + w],
                    )
                off += w

            for g in range(MT // 4):
                cur = gt
                if g + 1 < MT // 4:
                    gt = btp.tile([128, KT, 512], dt8)
                    for k in range(KT):
                        nc.sync.dma_start(
                            gt[:, k:k + 1, :],
                            bT_ap[128 * k:128 * (k + 1), 512 * (g + 1):512 * (g + 2)],
                        )
                for mm in range(4):
                    m = g * 4 + mm
                    sc = scp.tile([128, NG * GW], dth)
                    gmo = outp.tile([128, GM], dth)
                    # tail MMs first: fillers that need no psA slot
                    ptb = psB.tile([128, TAIL], dtf)
                    for p in range(2):
                        nc.tensor.matmul(
                            ptb[:],
                            cur[:, 2 * p:2 * p + 2, 128 * mm:128 * mm + 128],
                            emb8[:, 2 * p:2 * p + 2, NG * GW:VSP],
                            start=(p == 0),
                            stop=(p == 1),
                            perf_mode=DR,
                        )
                    for gi in range(NG):
                        off = gi * GW
                        pt = psA.tile([128, GW], dtf)
                        for p in range(2):
                            for c0 in range(0, GW, 512):
                                nc.tensor.matmul(
                                    pt[:, c0:c0 + 512],
                                    cur[:, 2 * p:2 * p + 2, 128 * mm:128 * mm + 128],
                                    emb8[:, 2 * p:2 * p + 2,
                                         off + c0:off + c0 + 512],
                                    start=(p == 0),
                                    stop=(p == 1),
                                    perf_mode=DR,
                                )
                        if gi == 0:
                            # early DVE slice balances the engines without
                            # delaying this slot's release
                            nc.vector.tensor_copy(sc[:, 0:DVE_CUT],
                                                  pt[:, 0:DVE_CUT])
                            nc.scalar.copy(sc[:, DVE_CUT:GW], pt[:, DVE_CUT:GW])
                        else:
                            nc.scalar.copy(sc[:, off:off + GW], pt[:])
                    nc.scalar.copy(gmo[:, 384:GM], ptb[:])
                    t1a = trp.tile([128, GW], dth)
                    nc.vector.tensor_max(t1a[:], sc[:, 0:GW], sc[:, GW:2 * GW])
                    t1b = trp.tile([128, GW], dth)
                    nc.vector.tensor_max(t1b[:], sc[:, 2 * GW:3 * GW],
                                         sc[:, 3 * GW:4 * GW])
                    q = trp.tile([128, GW], dth)
                    nc.vector.tensor_max(q[:], t1a[:], t1b[:])
                    t3 = trp.tile([128, 768], dth)
                    nc.vector.tensor_max(t3[:], q[:, 0:768], q[:, 768:GW])
                    nc.vector.tensor_max(gmo[:, 0:384], t3[:, 0:384],
                                         t3[:, 384:768])
                    # gm output rides the idle gpsimd queue so its long
                    # wait never blocks the bT prefetches on sync
                    nc.gpsimd.dma_start(gm_ap[128 * m:128 * (m + 1), :], gmo[:])
    nc.compile()
    return nc


def _run(batch: np.ndarray, emb: np.ndarray, trace: bool = False, **kw):
    import ml_dtypes
    from concourse import bass_utils

    if "nc" not in _CACHE:
        _CACHE["nc"] = _build()
    nc = _CACHE["nc"]
    f8 = ml_dtypes.float8_e4m3

    b = np.ascontiguousarray(batch.reshape(R, E).astype(np.float32))
    bT8 = np.ascontiguousarray(b.T).astype(f8)
    embT8 = emb.T.astype(f8)
    in_maps = []
    for c in range(NC):
        shardT = np.zeros((E, VSP), f8)
        shardT[:, :VS] = embT8[:, c * VS:(c + 1) * VS]
        in_maps.append({"bT": bT8, "embT": shardT})

    res = bass_utils.run_bass_kernel_spmd(
        nc, in_maps, core_ids=list(range(NC)), trace=trace, **kw
    )

    # gm: [R, NC*512] f16. Per core block: entries 0..383 are quad-tree
    # group maxima (group j covers local ids j + 384a + 1536b, a,b<4);
    # entries 384..511 are raw tail columns (local id 6144 + (p-384)).
    gm = np.concatenate([res.results[c]["gm"] for c in range(NC)], axis=1)
    gm = gm.astype(np.float32)                                     # [R, 4096]

    top = np.argpartition(-gm, TOPK, axis=1)[:, :TOPK]             # [R,K]
    core = top // GM
    p = top - core * GM                                            # [R,K]
    is_grp = p < 384
    ab = (384 * np.arange(4)[:, None] + 1536 * np.arange(4)[None, :]).reshape(-1)
    grp_cand = p[:, :, None] + ab[None, None, :]                   # [R,K,16]
    tail_cand = (6144 + (p - 384))[:, :, None] * np.ones(16, np.int64)
    loc = np.where(is_grp[:, :, None], grp_cand, tail_cand).reshape(R, -1)
    corex = np.repeat(core, 16, axis=1)
    valid = loc < VS
    gid = np.where(valid, corex * VS + np.minimum(loc, VS - 1), 0)

    # exact rescore in f32 on normalized embeddings
    en = emb / np.sqrt((emb * emb).sum(axis=1, keepdims=True))
    bn = b / np.sqrt((b * b).sum(axis=1, keepdims=True))
    best = np.empty(R, np.int64)
    CH = 256
    for r0 in range(0, R, CH):
        r1 = min(r0 + CH, R)
        g = gid[r0:r1]
        ce = en[g]                                   # [ch,K*16,512]
        s = np.matmul(ce, bn[r0:r1, :, None])[:, :, 0]
        s[~valid[r0:r1]] = -np.inf
        am = np.argmax(s, axis=1)
        best[r0:r1] = g[np.arange(r1 - r0), am]

    return best.astype(np.int32).reshape(B, S), res


def kernel(batch: np.ndarray, emb: np.ndarray) -> np.ndarray:
    out, _ = _run(batch, emb, trace=False)
    return out


# revision 18
# speedup vs baseline: 1.2480x; 1.2159x over previous
import numpy as np

# nn_NearestNeighbours: batch [8,512,512] f32, emb [50000,512] f32,
# output argmin indices [8,512] int32. Vocab-sharded across 8 cores.
# Each core screens 6144 of its 6250 vocab rows: fp8e4m3 DoubleRow
# GEMM in 3 psum groups of 2048 (2x 4-bank slots), ACT evicts to f16,
# DVE group-elementwise max tree (3-way, then two halvings) down to
# 512 group maxima per m-tile, DMA'd out on the idle gpsimd queue.
# The 106 leftover columns per core are scored exactly on the host
# (one small BLAS GEMM). Host picks global top-K groups, expands each
# to its 12 members (j + 512u + 2048v) and rescores exactly in f32.
B, S, E, V = 8, 512, 512, 50000
R = B * S              # 4096 token rows
NC = 8                 # cores
VS = V // NC           # 6250 vocab rows per core
VSP = 6144             # 3*2048 screened on device; tail 106 on host
GW = 2048              # psA group width
NG = 3                 # psA groups
GM = 512               # group maxima per m-tile row
TOPK = 20              # groups rescored exactly on host

_CACHE = {}


def _build():
    import concourse.bacc as bacc
    import concourse.mybir as mybir
    from concourse.tile import TileContext

    dtf = mybir.dt.float32
    dt8 = mybir.dt.float8e4
    dth = mybir.dt.float16
    DR = mybir.MatmulPerfMode.DoubleRow

    nc = bacc.Bacc("TRN2", target_bir_lowering=False, debug=False)
    bT_ap = nc.dram_tensor("bT", [E, R], dt8, kind="ExternalInput").ap()
    embT_ap = nc.dram_tensor("embT", [E, VSP], dt8, kind="ExternalInput").ap()
    gm_ap = nc.dram_tensor("gm", [R, GM], dth, kind="ExternalOutput").ap()

    KT = E // 128
    MT = R // 128
    with TileContext(nc) as tc:
        with tc.sbuf_pool(name="emb", bufs=1) as embp, \
             tc.sbuf_pool(name="bt", bufs=2) as btp, \
             tc.sbuf_pool(name="sc", bufs=2) as scp, \
             tc.sbuf_pool(name="tr", bufs=2) as trp, \
             tc.sbuf_pool(name="out", bufs=4) as outp, \
             tc.psum_pool(name="psA", bufs=2) as psA:
            gt = btp.tile([128, KT, 512], dt8)
            for k in range(KT):
                nc.scalar.dma_start(gt[:, k:k + 1, :],
                                    bT_ap[128 * k:128 * (k + 1), 0:512])
            emb8 = embp.tile([128, KT, VSP], dt8, name="emb8")
            off = 0
            for w in [1024] * 6:
                eng = nc.scalar if off >= 5120 else nc.sync
                for k in range(KT):
                    eng.dma_start(
                        emb8[:, k:k + 1, off:off + w],
                        embT_ap[128 * k:128 * (k + 1), off:off + w],
                    )
                off += w

            for g in range(MT // 4):
                cur = gt
                if g + 1 < MT // 4:
                    gt = btp.tile([128, KT, 512], dt8)
                    for k in range(KT):
                        nc.sync.dma_start(
                            gt[:, k:k + 1, :],
                            bT_ap[128 * k:128 * (k + 1), 512 * (g + 1):512 * (g + 2)],
                        )
                for mm in range(4):
                    m = g * 4 + mm
                    sc = scp.tile([128, VSP], dth)
                    gmo = outp.tile([128, GM], dth)
                    for gi in range(NG):
                        off = gi * GW
                        pt = psA.tile([128, GW], dtf)
                        for p in range(2):
                            for c0 in range(0, GW, 512):
                                nc.tensor.matmul(
                                    pt[:, c0:c0 + 512],
                                    cur[:, 2 * p:2 * p + 2, 128 * mm:128 * mm + 128],
                                    emb8[:, 2 * p:2 * p + 2,
                                         off + c0:off + c0 + 512],
                                    start=(p == 0),
                                    stop=(p == 1),
                                    perf_mode=DR,
                                )
                        nc.scalar.copy(sc[:, off:off + GW], pt[:])
                    t1 = trp.tile([128, GW], dth)
                    nc.vector.tensor_max(t1[:], sc[:, 0:GW], sc[:, GW:2 * GW])
                    q = trp.tile([128, GW], dth)
                    nc.vector.tensor_max(q[:], t1[:], sc[:, 2 * GW:3 * GW])
                    r_ = trp.tile([128, 1024], dth)
                    nc.vector.tensor_max(r_[:], q[:, 0:1024], q[:, 1024:GW])
                    nc.vector.tensor_max(gmo[:], r_[:, 0:512], r_[:, 512:1024])
                    # gm output rides the idle gpsimd queue so its long
                    # wait never blocks the bT prefetches on sync
                    nc.gpsimd.dma_start(gm_ap[128 * m:128 * (m + 1), :], gmo[:])
    nc.compile()
    return nc


def _run(batch: np.ndarray, emb: np.ndarray, trace: bool = False, **kw):
    import ml_dtypes
    from concourse import bass_utils

    if "nc" not in _CACHE:
        _CACHE["nc"] = _build()
    nc = _CACHE["nc"]
    f8 = ml_dtypes.float8_e4m3

    b = np.ascontiguousarray(batch.reshape(R, E).astype(np.float32))
    bT8 = np.ascontiguousarray(b.T).astype(f8)
    embT8 = emb.T.astype(f8)
    in_maps = []
    for c in range(NC):
        in_maps.append({
            "bT": bT8,
            "embT": np.ascontiguousarray(embT8[:, c * VS:c * VS + VSP]),
        })

    res = bass_utils.run_bass_kernel_spmd(
        nc, in_maps, core_ids=list(range(NC)), trace=trace, **kw
    )

    # gm: [R, NC*512] f16; group (c, j) covers local ids j + 512u + 2048v
    gm = np.concatenate([res.results[c]["gm"] for c in range(NC)], axis=1)
    gm = gm.astype(np.float32)                                     # [R, 4096]

    top = np.argpartition(-gm, TOPK, axis=1)[:, :TOPK]             # [R,K]
    core = top // GM
    j = top - core * GM
    uv = (512 * np.arange(4)[:, None] + 2048 * np.arange(3)[None, :]).reshape(-1)
    loc = (j[:, :, None] + uv[None, None, :]).reshape(R, -1)       # [R,K*12]
    gid = np.repeat(core, 12, axis=1) * VS + loc

    # exact rescore in f32 on normalized embeddings
    en = emb / np.sqrt((emb * emb).sum(axis=1, keepdims=True))
    bn = b / np.sqrt((b * b).sum(axis=1, keepdims=True))
    best = np.empty(R, np.int64)
    bestv = np.empty(R, np.float32)
    CH = 256
    for r0 in range(0, R, CH):
        r1 = min(r0 + CH, R)
        g = gid[r0:r1]
        ce = en[g]                                   # [ch,K*12,512]
        s = np.matmul(ce, bn[r0:r1, :, None])[:, :, 0]
        am = np.argmax(s, axis=1)
        ar = np.arange(r1 - r0)
        best[r0:r1] = g[ar, am]
        bestv[r0:r1] = s[ar, am]

    # the 106 tail columns per core, scored exactly on host
    tail_ids = np.concatenate(
        [np.arange(c * VS + VSP, (c + 1) * VS) for c in range(NC)])
    ts = bn @ en[tail_ids].T                         # [R, 848]
    tam = np.argmax(ts, axis=1)
    ar = np.arange(R)
    tbest = tail_ids[tam]
    tv = ts[ar, tam]
    use_tail = tv > bestv
    best = np.where(use_tail, tbest, best)

    return best.astype(np.int32).reshape(B, S), res


def kernel(batch: np.ndarray, emb: np.ndarray) -> np.ndarray:
    out, _ = _run(batch, emb, trace=False)
    return out
